# revision 17
# baseline (speedup 1.0000x reference)
"""CausalTemporalAttnBlock Trainium2 kernel (v3).

Problem: out = x + Wp @ attn(norm(x)) + bp, where norm is GroupNorm(1 group)
over (c,t,h,w) per batch, attention is causal over t, independent per (b,h,w).
Shapes: x (2, 512, 64, 32, 32) fp32; four (512,512) weights + biases.

Strategy (8 NeuronCores, zero device-to-device communication):
  - core i handles batch i//4, h-rows [8*(i%4), 8*(i%4)+8), all w.
  - GroupNorm stats (mean/rstd per batch) are computed on host in float64 and
    folded, with gamma/beta and the attention scale, into the projections.
  - Associativity folding collapses the four (c,c) projections into TWO
    device GEMMs (folds are host-side float64 on (512,512) matrices):
      * scores: k_s . q_t = x_s^T (Ak^T Aq) x_t + (Ak^T cq) . x_s + terms
        that are constant along s and cancel in softmax. So Y = G x + u with
        G = Ak^T Aq, and S^T = x^T Y.
      * output: x + Wp(sum_s a_s v_s) + bp = x + sum_s a_s (M x_s) + cvt,
        with M = Wp Av, cvt = Wp cv + bp (softmax weights sum to 1 exactly).
  - Everything ships and computes in bf16 (PE runs bf16 at 1 cycle/row for
    any moving size, vs fp32r's 4x penalty below 256 columns; DMA bytes and
    SBUF footprint halve). PSUM accumulation stays fp32; biases applied
    during PSUM eviction.
  - Attention processes (h,w) locations in PAIRS packed into the 128-wide
    partition dim: scores for a pair are [128,128] matmuls per ci chunk
    (cross-location junk is zeroed by the mask multiply before the softmax
    column sums), (Mx)^T for a pair is produced at full PE rate with the x
    slab [c,128] stationary, and AV contracts the full 128 partitions.
  - Multi-chain PSUM banks rely on PE program order: the first matmul of a
    bank runs start=True (zeroing the whole bank), later chains accumulate
    with start=False. No DVE memsets.
  - Eviction/elementwise ops are emitted on nc.any and load-balanced across
    DVE/ACT by the scheduler.
"""

import numpy as np

import concourse.bass as bass
import concourse.tile as tile
from concourse import bacc, mybir
from concourse.bass_utils import run_bass_kernel_spmd

P = 128
B, C, T, H, W = 2, 512, 64, 32, 32
NCORES = 8
HSH = H // 4          # 8 h-rows per core
CCH = C // P          # 4 c chunks
GRP = 8               # w locations per group (4 pairs)
NGRP = W // GRP       # 4 groups per block
NPAIR = GRP // 2
GCOL = GRP * T        # 512 columns per group
EPS = 1e-6

f32 = mybir.dt.float32
bf16 = mybir.dt.bfloat16
ALU = mybir.AluOpType
AF = mybir.ActivationFunctionType
NPBF16 = mybir.dt.np(bf16)


def build_nc(num_cores=NCORES, nblk=HSH):
    nc = bacc.Bacc("TRN2", target_bir_lowering=False, debug=False,
                   num_devices=num_cores)

    xs = nc.declare_dram_parameter("xs", [C, nblk * W * T], bf16,
                                   isOutput=False)
    gt = nc.declare_dram_parameter("gt", [C, C], bf16, isOutput=False)
    mt = nc.declare_dram_parameter("mt", [C, C], bf16, isOutput=False)
    bcol = nc.declare_dram_parameter("bcol", [P, 2 * CCH], f32, isOutput=False)
    maskp = nc.declare_dram_parameter("maskt", [P, GCOL], bf16, isOutput=False)
    onescol = nc.declare_dram_parameter("onescol", [P, 1], bf16, isOutput=False)
    onesrow = nc.declare_dram_parameter("onesrow", [1, P], bf16, isOutput=False)
    outp = nc.declare_dram_parameter("out", [C, nblk * W * T], bf16,
                                     isOutput=True)

    with tile.TileContext(nc) as tc:
        with (
            tc.tile_pool(name="const", bufs=1) as const,
            tc.tile_pool(name="xpool", bufs=2) as xpool,
            tc.tile_pool(name="ypool", bufs=2) as ypool,
            tc.tile_pool(name="vpool", bufs=2) as vpool,
            tc.tile_pool(name="spool", bufs=2) as spool,
            tc.tile_pool(name="py", bufs=2, space="PSUM") as py,
            tc.tile_pool(name="pav", bufs=2, space="PSUM") as pav,
            tc.tile_pool(name="pss", bufs=2, space="PSUM") as pss,
            tc.tile_pool(name="psm", bufs=1, space="PSUM") as psm,
            tc.tile_pool(name="psx", bufs=1, space="PSUM") as psx,
        ):
            # ---------- constants ----------
            g_sb, m_sb = [], []
            for ci in range(CCH):
                t = const.tile([P, C], bf16, tag=f"g{ci}")
                nc.sync.dma_start(t[:], gt[ci * P:(ci + 1) * P, :])
                g_sb.append(t)
                t = const.tile([P, C], bf16, tag=f"m{ci}")
                nc.sync.dma_start(t[:], mt[ci * P:(ci + 1) * P, :])
                m_sb.append(t)
            bcol_sb = const.tile([P, 2 * CCH], f32, tag="bcol")
            nc.sync.dma_start(bcol_sb[:], bcol[:])
            mask_sb = const.tile([P, GCOL], bf16, tag="maskt")
            nc.sync.dma_start(mask_sb[:], maskp[:])
            oc_sb = const.tile([P, 1], bf16, tag="oc")
            nc.sync.dma_start(oc_sb[:], onescol[:])
            or_sb = const.tile([1, P], bf16, tag="or")
            nc.sync.dma_start(or_sb[:], onesrow[:])

            with nc.allow_low_precision(reason="bf16 within rel-err budget"):
                for blk in range(nblk):
                    xb = []
                    for ci in range(CCH):
                        t = xpool.tile([P, W * T], bf16, tag=f"xb{ci}")
                        nc.sync.dma_start(
                            t[:], xs[ci * P:(ci + 1) * P,
                                     blk * W * T:(blk + 1) * W * T])
                        xb.append(t)

                    for g in range(NGRP):
                        c0 = g * GCOL

                        # ---- Y = G x + u (replaces Q and K projections) ----
                        yg = []
                        for co in range(CCH):
                            ps = py.tile([P, GCOL], f32, tag="py")
                            for ci in range(CCH):
                                nc.tensor.matmul(
                                    ps[:],
                                    g_sb[ci][:, co * P:(co + 1) * P],
                                    xb[ci][:, c0:c0 + GCOL],
                                    start=(ci == 0), stop=(ci == CCH - 1))
                            t = ypool.tile([P, GCOL], bf16, tag=f"y{co}")
                            nc.any.tensor_scalar(
                                t[:], ps[:], bcol_sb[:, co:co + 1], None,
                                ALU.add)
                            yg.append(t)

                        # ---- (M x)^T per pair: [128 (2 locs x t), 512 co] ----
                        vt = []
                        for p in range(NPAIR):
                            ps = pss.tile([P, C], f32, tag="ppv")
                            for ci in range(CCH):
                                nc.tensor.matmul(
                                    ps[:],
                                    xb[ci][:, c0 + p * P:c0 + (p + 1) * P],
                                    m_sb[ci][:],
                                    start=(ci == 0), stop=(ci == CCH - 1))
                            t = vpool.tile([P, C], bf16, tag=f"vt{p}")
                            nc.any.tensor_copy(t[:], ps[:])
                            vt.append(t)

                        # ---- scores S^T[s01, (pair, t01)] = x^T Y ----
                        # first matmul start=True zeroes the whole PSUM bank;
                        # later pairs accumulate start=False onto those zeros
                        # (PE executes its queue in program order)
                        ps_s = psm.tile([P, GCOL], f32, tag="pss")
                        for p in range(NPAIR):
                            for ci in range(CCH):
                                xl = xb[ci][:, c0 + p * P:c0 + (p + 1) * P]
                                nc.tensor.matmul(
                                    ps_s[:, p * P:(p + 1) * P],
                                    xl,
                                    yg[ci][:, p * P:(p + 1) * P],
                                    start=(p == 0 and ci == 0),
                                    stop=(ci == CCH - 1),
                                    skip_group_check=True)
                        # ---- softmax (no max subtraction; scores O(1)) ----
                        pexp = spool.tile([P, GCOL], bf16, tag="pexp")
                        nc.scalar.activation(pexp[:], ps_s[:], AF.Exp)
                        pm = spool.tile([P, GCOL], bf16, tag="pmask")
                        nc.any.tensor_mul(pm[:], pexp[:], mask_sb[:])
                        ps_x = psx.tile([P, GCOL], f32, tag="psx")
                        nc.tensor.matmul(ps_x[0:1, :], oc_sb[:], pm[:],
                                         start=True, stop=True)
                        rs = spool.tile([1, GCOL], bf16, tag="rs")
                        nc.vector.reciprocal(rs[:], ps_x[0:1, :])
                        nc.tensor.matmul(ps_x[:], or_sb[:], rs[:],
                                         start=True, stop=True)
                        pn = spool.tile([P, GCOL], bf16, tag="pn")
                        nc.any.tensor_mul(pn[:], pm[:], ps_x[:])

                        # ---- AV (already P-projected): out rows co ----
                        for co in range(CCH):
                            ps_o = pav.tile([P, GCOL], f32, tag="pav")
                            for p in range(NPAIR):
                                nc.tensor.matmul(
                                    ps_o[:, p * P:(p + 1) * P],
                                    vt[p][:, co * P:(co + 1) * P],
                                    pn[:, p * P:(p + 1) * P],
                                    start=(p == 0), stop=(p == NPAIR - 1),
                                    skip_group_check=True)
                            # cvt bias rides here (softmax weights sum to 1),
                            # then the residual add into x
                            t = spool.tile([P, GCOL], bf16, tag="ot")
                            nc.any.tensor_scalar(
                                t[:], ps_o[:], bcol_sb[:, CCH + co:CCH + co + 1],
                                None, ALU.add)
                            nc.any.tensor_add(xb[co][:, c0:c0 + GCOL],
                                              t[:], xb[co][:, c0:c0 + GCOL])

                    for ci in range(CCH):
                        nc.sync.dma_start(
                            outp[ci * P:(ci + 1) * P,
                                 blk * W * T:(blk + 1) * W * T], xb[ci][:])
    nc.compile()
    return nc


def host_prep(x, gamma, beta, wq, bq, wk, bk, wv, bv, wp, bp):
    """Fold GroupNorm stats, gamma/beta, the attention scale, and the Q/K and
    V/P projection pairs into two (c,c) matrices + biases per batch."""
    s = 1.0 / np.sqrt(np.float64(C))
    n = C * T * H * W
    g64 = gamma.astype(np.float64)
    b64 = beta.astype(np.float64)

    per_batch = []
    for b in range(B):
        xf = x[b].reshape(-1)
        s1 = float(xf.sum(dtype=np.float64))
        s2 = float(np.dot(xf, xf))
        mu = s1 / n
        var = s2 / n - mu * mu
        r = 1.0 / np.sqrt(var + EPS)
        gp = g64 * r                       # per-channel scale on x
        cb = b64 - mu * gp                 # per-channel offset

        def fold(w, bias, scale):
            w64 = w.astype(np.float64)
            a = (w64 * gp[None, :]) * scale          # (co, ci)
            c0 = (bias.astype(np.float64) + w64 @ cb) * scale
            return a, c0

        aq, cq = fold(wq, bq, s)
        ak, ck = fold(wk, bk, 1.0)
        av, cv = fold(wv, bv, 1.0)
        wp64 = wp.astype(np.float64)

        G = ak.T @ aq                      # (ci_s -> scores via x^T G x)
        u = ak.T @ cq                      # row bias (varies along s)
        M = wp64 @ av                      # fused V+P projection
        cvt = wp64 @ cv + bp.astype(np.float64)

        bcol = np.empty((P, 2 * CCH), np.float32)
        for ch in range(CCH):
            bcol[:, ch] = u[ch * P:(ch + 1) * P]
            bcol[:, CCH + ch] = cvt[ch * P:(ch + 1) * P]

        per_batch.append({
            "gt": np.ascontiguousarray(G.T).astype(NPBF16),
            "mt": np.ascontiguousarray(M.T).astype(NPBF16),
            "bcol": bcol,
        })

    # mask for one pair block [128, 128]: blockdiag of two causal(64) masks,
    # [s, t] keep s <= t; tiled across the 4 pairs of a group
    tri = np.triu(np.ones((T, T), np.float32))
    blk = np.zeros((P, P), np.float32)
    blk[:T, :T] = tri
    blk[T:, T:] = tri
    maskt = np.tile(blk, (1, NPAIR)).astype(NPBF16)
    shared = {
        "maskt": np.ascontiguousarray(maskt),
        "onescol": np.ones((P, 1), NPBF16),
        "onesrow": np.ones((1, P), NPBF16),
    }
    return per_batch, shared


_NC_CACHE = {}


def kernel(x, gamma, beta, wq, bq, wk, bk, wv, bv, wp, bp):
    x = np.asarray(x, np.float32)
    args = [np.asarray(a, np.float32) for a in
            (gamma, beta, wq, bq, wk, bk, wv, bv, wp, bp)]
    per_batch, shared = host_prep(x, *args)

    if "nc" not in _NC_CACHE:
        _NC_CACHE["nc"] = build_nc()
    nc = _NC_CACHE["nc"]

    in_maps = []
    for core in range(NCORES):
        b, hg = core // 4, core % 4
        shard = x[b, :, :, hg * HSH:(hg + 1) * HSH, :]        # (C,T,HSH,W)
        shard = np.ascontiguousarray(
            shard.transpose(0, 2, 3, 1)).reshape(C, HSH * W * T)
        in_maps.append({"xs": shard.astype(NPBF16),
                        **per_batch[b], **shared})

    global _last_in_maps
    _last_in_maps = in_maps
    res = run_bass_kernel_spmd(nc, in_maps, list(range(NCORES)))

    out = np.empty((B, C, T, H, W), np.float32)
    for core in range(NCORES):
        b, hg = core // 4, core % 4
        o = res.results[core]["out"].astype(np.float32)
        o = o.reshape(C, HSH, W, T)
        out[b, :, :, hg * HSH:(hg + 1) * HSH, :] = o.transpose(0, 3, 1, 2)
    return out


# revision 19
# speedup vs baseline: 1.0055x; 1.0055x over previous
"""CausalTemporalAttnBlock Trainium2 kernel (v3).

Problem: out = x + Wp @ attn(norm(x)) + bp, where norm is GroupNorm(1 group)
over (c,t,h,w) per batch, attention is causal over t, independent per (b,h,w).
Shapes: x (2, 512, 64, 32, 32) fp32; four (512,512) weights + biases.

Strategy (8 NeuronCores, zero device-to-device communication):
  - core i handles batch i//4, h-rows [8*(i%4), 8*(i%4)+8), all w.
  - GroupNorm stats (mean/rstd per batch) are computed on host in float64 and
    folded, with gamma/beta and the attention scale, into the projections.
  - Associativity folding collapses the four (c,c) projections into TWO
    device GEMMs (folds are host-side float64 on (512,512) matrices):
      * scores: k_s . q_t = x_s^T (Ak^T Aq) x_t + (Ak^T cq) . x_s + terms
        that are constant along s and cancel in softmax. So Y = G x + u with
        G = Ak^T Aq, and S^T = x^T Y.
      * output: x + Wp(sum_s a_s v_s) + bp = x + sum_s a_s (M x_s) + cvt,
        with M = Wp Av, cvt = Wp cv + bp (softmax weights sum to 1 exactly).
  - Everything ships and computes in bf16 (PE runs bf16 at 1 cycle/row for
    any moving size, vs fp32r's 4x penalty below 256 columns; DMA bytes and
    SBUF footprint halve). PSUM accumulation stays fp32; biases applied
    during PSUM eviction.
  - Attention processes (h,w) locations in PAIRS packed into the 128-wide
    partition dim: scores for a pair are [128,128] matmuls per ci chunk
    (cross-location junk is zeroed by the mask multiply before the softmax
    column sums), (Mx)^T for a pair is produced at full PE rate with the x
    slab [c,128] stationary, and AV contracts the full 128 partitions.
  - Multi-chain PSUM banks rely on PE program order: the first matmul of a
    bank runs start=True (zeroing the whole bank), later chains accumulate
    with start=False. No DVE memsets.
  - Eviction/elementwise ops are emitted on nc.any and load-balanced across
    DVE/ACT by the scheduler.
"""

import numpy as np

import concourse.bass as bass
import concourse.tile as tile
from concourse import bacc, mybir
from concourse.bass_utils import run_bass_kernel_spmd

P = 128
B, C, T, H, W = 2, 512, 64, 32, 32
NCORES = 8
HSH = H // 4          # 8 h-rows per core
CCH = C // P          # 4 c chunks
GRP = 8               # w locations per group (4 pairs)
NGRP = W // GRP       # 4 groups per block
NPAIR = GRP // 2
GCOL = GRP * T        # 512 columns per group
EPS = 1e-6

f32 = mybir.dt.float32
bf16 = mybir.dt.bfloat16
ALU = mybir.AluOpType
AF = mybir.ActivationFunctionType
NPBF16 = mybir.dt.np(bf16)


def build_nc(num_cores=NCORES, nblk=HSH):
    nc = bacc.Bacc("TRN2", target_bir_lowering=False, debug=False,
                   num_devices=num_cores)

    xs = nc.declare_dram_parameter("xs", [C, nblk * W * T], bf16,
                                   isOutput=False)
    gt = nc.declare_dram_parameter("gt", [C, C], bf16, isOutput=False)
    mt = nc.declare_dram_parameter("mt", [C, C], bf16, isOutput=False)
    bcol = nc.declare_dram_parameter("bcol", [P, 2 * CCH], f32, isOutput=False)
    maskp = nc.declare_dram_parameter("maskt", [P, GCOL], bf16, isOutput=False)
    onescol = nc.declare_dram_parameter("onescol", [P, 1], bf16, isOutput=False)
    onesrow = nc.declare_dram_parameter("onesrow", [1, P], bf16, isOutput=False)
    outp = nc.declare_dram_parameter("out", [C, nblk * W * T], bf16,
                                     isOutput=True)

    with tile.TileContext(nc) as tc:
        with (
            tc.tile_pool(name="const", bufs=1) as const,
            tc.tile_pool(name="xpool", bufs=2) as xpool,
            tc.tile_pool(name="ypool", bufs=2) as ypool,
            tc.tile_pool(name="vpool", bufs=2) as vpool,
            tc.tile_pool(name="spool", bufs=2) as spool,
            tc.tile_pool(name="py", bufs=2, space="PSUM") as py,
            tc.tile_pool(name="pav", bufs=2, space="PSUM") as pav,
            tc.tile_pool(name="pss", bufs=2, space="PSUM") as pss,
            tc.tile_pool(name="psm", bufs=1, space="PSUM") as psm,
            tc.tile_pool(name="psx", bufs=1, space="PSUM") as psx,
        ):
            # ---------- constants (small ones first; G before M so the
            # first Y matmuls can start while M/mask still stream in) ----------
            bcol_sb = const.tile([P, 2 * CCH], f32, tag="bcol")
            nc.sync.dma_start(bcol_sb[:], bcol[:])
            oc_sb = const.tile([P, 1], bf16, tag="oc")
            nc.sync.dma_start(oc_sb[:], onescol[:])
            or_sb = const.tile([1, P], bf16, tag="or")
            nc.sync.dma_start(or_sb[:], onesrow[:])
            g_sb, m_sb = [], []
            for ci in range(CCH):
                t = const.tile([P, C], bf16, tag=f"g{ci}")
                nc.sync.dma_start(t[:], gt[ci * P:(ci + 1) * P, :])
                g_sb.append(t)
            for ci in range(CCH):
                t = const.tile([P, C], bf16, tag=f"m{ci}")
                nc.sync.dma_start(t[:], mt[ci * P:(ci + 1) * P, :])
                m_sb.append(t)
            mask_sb = const.tile([P, GCOL], bf16, tag="maskt")
            nc.sync.dma_start(mask_sb[:], maskp[:])

            with nc.allow_low_precision(reason="bf16 within rel-err budget"):

                def emit_dma_in(blk):
                    xb = []
                    for ci in range(CCH):
                        t = xpool.tile([P, W * T], bf16, tag=f"xb{ci}")
                        nc.sync.dma_start(
                            t[:], xs[ci * P:(ci + 1) * P,
                                     blk * W * T:(blk + 1) * W * T])
                        xb.append(t)
                    return xb

                def emit_a(xb, g):
                    """Y = G x + u projection and (M x)^T per pair."""
                    c0 = g * GCOL
                    yg = []
                    for co in range(CCH):
                        ps = py.tile([P, GCOL], f32, tag="py")
                        for ci in range(CCH):
                            nc.tensor.matmul(
                                ps[:], g_sb[ci][:, co * P:(co + 1) * P],
                                xb[ci][:, c0:c0 + GCOL],
                                start=(ci == 0), stop=(ci == CCH - 1))
                        t = ypool.tile([P, GCOL], bf16, tag=f"y{co}")
                        nc.any.tensor_scalar(t[:], ps[:],
                                             bcol_sb[:, co:co + 1], None,
                                             ALU.add)
                        yg.append(t)
                    vt = []
                    for p in range(NPAIR):
                        ps = pss.tile([P, C], f32, tag="ppv")
                        for ci in range(CCH):
                            nc.tensor.matmul(
                                ps[:], xb[ci][:, c0 + p * P:c0 + (p + 1) * P],
                                m_sb[ci][:],
                                start=(ci == 0), stop=(ci == CCH - 1))
                        t = vpool.tile([P, C], bf16, tag=f"vt{p}")
                        nc.any.tensor_copy(t[:], ps[:])
                        vt.append(t)
                    return yg, vt

                def emit_scores(xb, g, yg):
                    # first matmul start=True zeroes the whole PSUM bank;
                    # later pairs accumulate start=False onto those zeros
                    # (PE executes its queue in program order)
                    c0 = g * GCOL
                    ps_s = psm.tile([P, GCOL], f32, tag="pss")
                    for p in range(NPAIR):
                        for ci in range(CCH):
                            xl = xb[ci][:, c0 + p * P:c0 + (p + 1) * P]
                            nc.tensor.matmul(
                                ps_s[:, p * P:(p + 1) * P], xl,
                                yg[ci][:, p * P:(p + 1) * P],
                                start=(p == 0 and ci == 0),
                                stop=(ci == CCH - 1),
                                skip_group_check=True)
                    return ps_s

                def emit_b(xb, g, ps_s, vt):
                    """Softmax (no max subtraction; scores O(1)), AV (already
                    P-projected via M), cvt bias (softmax weights sum to 1),
                    residual into x in place."""
                    c0 = g * GCOL
                    pexp = spool.tile([P, GCOL], bf16, tag="pexp")
                    nc.scalar.activation(pexp[:], ps_s[:], AF.Exp)
                    pm = spool.tile([P, GCOL], bf16, tag="pmask")
                    nc.any.tensor_mul(pm[:], pexp[:], mask_sb[:])
                    ps_x = psx.tile([P, GCOL], f32, tag="psx")
                    nc.tensor.matmul(ps_x[0:1, :], oc_sb[:], pm[:],
                                     start=True, stop=True)
                    rs = spool.tile([1, GCOL], bf16, tag="rs")
                    nc.vector.reciprocal(rs[:], ps_x[0:1, :])
                    nc.tensor.matmul(ps_x[:], or_sb[:], rs[:],
                                     start=True, stop=True)
                    pn = spool.tile([P, GCOL], bf16, tag="pn")
                    nc.any.tensor_mul(pn[:], pm[:], ps_x[:])
                    for co in range(CCH):
                        ps_o = pav.tile([P, GCOL], f32, tag="pav")
                        for p in range(NPAIR):
                            nc.tensor.matmul(
                                ps_o[:, p * P:(p + 1) * P],
                                vt[p][:, co * P:(co + 1) * P],
                                pn[:, p * P:(p + 1) * P],
                                start=(p == 0), stop=(p == NPAIR - 1),
                                skip_group_check=True)
                        t = spool.tile([P, GCOL], bf16, tag="ot")
                        nc.any.tensor_scalar(
                            t[:], ps_o[:],
                            bcol_sb[:, CCH + co:CCH + co + 1], None, ALU.add)
                        nc.any.tensor_add(xb[co][:, c0:c0 + GCOL], t[:],
                                          xb[co][:, c0:c0 + GCOL])

                # software pipeline: A(next) is emitted between scores(cur)
                # and the softmax-dependent PE work of cur, so the in-order
                # PE queue never blocks on ACT/DVE evictions
                xb_cur = emit_dma_in(0)
                state = emit_a(xb_cur, 0)
                xb_next = None
                for blk in range(nblk):
                    if blk + 1 < nblk:
                        xb_next = emit_dma_in(blk + 1)
                    for g in range(NGRP):
                        ps_s = emit_scores(xb_cur, g, state[0])
                        vt_cur = state[1]
                        if g + 1 < NGRP:
                            state = emit_a(xb_cur, g + 1)
                        elif blk + 1 < nblk:
                            state = emit_a(xb_next, 0)
                        else:
                            state = None
                        emit_b(xb_cur, g, ps_s, vt_cur)
                    if blk + 1 < nblk:
                        for ci in range(CCH):
                            nc.sync.dma_start(
                                outp[ci * P:(ci + 1) * P,
                                     blk * W * T:(blk + 1) * W * T],
                                xb_cur[ci][:])
                    else:
                        # drain tail: ship the last block per group so the
                        # final DMA after the last compute is only 1/4 block
                        for g in range(NGRP):
                            for ci in range(CCH):
                                o0 = blk * W * T + g * GCOL
                                nc.sync.dma_start(
                                    outp[ci * P:(ci + 1) * P, o0:o0 + GCOL],
                                    xb_cur[ci][:, g * GCOL:(g + 1) * GCOL])
                    xb_cur = xb_next
    nc.compile()
    return nc


def host_prep(x, gamma, beta, wq, bq, wk, bk, wv, bv, wp, bp):
    """Fold GroupNorm stats, gamma/beta, the attention scale, and the Q/K and
    V/P projection pairs into two (c,c) matrices + biases per batch."""
    s = 1.0 / np.sqrt(np.float64(C))
    n = C * T * H * W
    g64 = gamma.astype(np.float64)
    b64 = beta.astype(np.float64)

    per_batch = []
    for b in range(B):
        xf = x[b].reshape(-1)
        s1 = float(xf.sum(dtype=np.float64))
        s2 = float(np.dot(xf, xf))
        mu = s1 / n
        var = s2 / n - mu * mu
        r = 1.0 / np.sqrt(var + EPS)
        gp = g64 * r                       # per-channel scale on x
        cb = b64 - mu * gp                 # per-channel offset

        def fold(w, bias, scale):
            w64 = w.astype(np.float64)
            a = (w64 * gp[None, :]) * scale          # (co, ci)
            c0 = (bias.astype(np.float64) + w64 @ cb) * scale
            return a, c0

        aq, cq = fold(wq, bq, s)
        ak, ck = fold(wk, bk, 1.0)
        av, cv = fold(wv, bv, 1.0)
        wp64 = wp.astype(np.float64)

        G = ak.T @ aq                      # (ci_s -> scores via x^T G x)
        u = ak.T @ cq                      # row bias (varies along s)
        M = wp64 @ av                      # fused V+P projection
        cvt = wp64 @ cv + bp.astype(np.float64)

        bcol = np.empty((P, 2 * CCH), np.float32)
        for ch in range(CCH):
            bcol[:, ch] = u[ch * P:(ch + 1) * P]
            bcol[:, CCH + ch] = cvt[ch * P:(ch + 1) * P]

        per_batch.append({
            "gt": np.ascontiguousarray(G.T).astype(NPBF16),
            "mt": np.ascontiguousarray(M.T).astype(NPBF16),
            "bcol": bcol,
        })

    # mask for one pair block [128, 128]: blockdiag of two causal(64) masks,
    # [s, t] keep s <= t; tiled across the 4 pairs of a group
    tri = np.triu(np.ones((T, T), np.float32))
    blk = np.zeros((P, P), np.float32)
    blk[:T, :T] = tri
    blk[T:, T:] = tri
    maskt = np.tile(blk, (1, NPAIR)).astype(NPBF16)
    shared = {
        "maskt": np.ascontiguousarray(maskt),
        "onescol": np.ones((P, 1), NPBF16),
        "onesrow": np.ones((1, P), NPBF16),
    }
    return per_batch, shared


_NC_CACHE = {}


def kernel(x, gamma, beta, wq, bq, wk, bk, wv, bv, wp, bp):
    x = np.asarray(x, np.float32)
    args = [np.asarray(a, np.float32) for a in
            (gamma, beta, wq, bq, wk, bk, wv, bv, wp, bp)]
    per_batch, shared = host_prep(x, *args)

    if "nc" not in _NC_CACHE:
        _NC_CACHE["nc"] = build_nc()
    nc = _NC_CACHE["nc"]

    in_maps = []
    for core in range(NCORES):
        b, hg = core // 4, core % 4
        shard = x[b, :, :, hg * HSH:(hg + 1) * HSH, :]        # (C,T,HSH,W)
        shard = np.ascontiguousarray(
            shard.transpose(0, 2, 3, 1)).reshape(C, HSH * W * T)
        in_maps.append({"xs": shard.astype(NPBF16),
                        **per_batch[b], **shared})

    global _last_in_maps
    _last_in_maps = in_maps
    res = run_bass_kernel_spmd(nc, in_maps, list(range(NCORES)))

    out = np.empty((B, C, T, H, W), np.float32)
    for core in range(NCORES):
        b, hg = core // 4, core % 4
        o = res.results[core]["out"].astype(np.float32)
        o = o.reshape(C, HSH, W, T)
        out[b, :, :, hg * HSH:(hg + 1) * HSH, :] = o.transpose(0, 3, 1, 2)
    return out


# revision 20
# speedup vs baseline: 1.0196x; 1.0141x over previous
"""CausalTemporalAttnBlock Trainium2 kernel (v3).

Problem: out = x + Wp @ attn(norm(x)) + bp, where norm is GroupNorm(1 group)
over (c,t,h,w) per batch, attention is causal over t, independent per (b,h,w).
Shapes: x (2, 512, 64, 32, 32) fp32; four (512,512) weights + biases.

Strategy (8 NeuronCores, zero device-to-device communication):
  - core i handles batch i//4, h-rows [8*(i%4), 8*(i%4)+8), all w.
  - GroupNorm stats (mean/rstd per batch) are computed on host in float64 and
    folded, with gamma/beta and the attention scale, into the projections.
  - Associativity folding collapses the four (c,c) projections into TWO
    device GEMMs (folds are host-side float64 on (512,512) matrices):
      * scores: k_s . q_t = x_s^T (Ak^T Aq) x_t + (Ak^T cq) . x_s + terms
        that are constant along s and cancel in softmax. So Y = G x + u with
        G = Ak^T Aq, and S^T = x^T Y.
      * output: x + Wp(sum_s a_s v_s) + bp = x + sum_s a_s (M x_s) + cvt,
        with M = Wp Av, cvt = Wp cv + bp (softmax weights sum to 1 exactly).
  - Everything ships and computes in bf16 (PE runs bf16 at 1 cycle/row for
    any moving size, vs fp32r's 4x penalty below 256 columns; DMA bytes and
    SBUF footprint halve). PSUM accumulation stays fp32; biases applied
    during PSUM eviction.
  - Attention processes (h,w) locations in PAIRS packed into the 128-wide
    partition dim: scores for a pair are [128,128] matmuls per ci chunk
    (cross-location junk is zeroed by the mask multiply before the softmax
    column sums), (Mx)^T for a pair is produced at full PE rate with the x
    slab [c,128] stationary, and AV contracts the full 128 partitions.
  - Multi-chain PSUM banks rely on PE program order: the first matmul of a
    bank runs start=True (zeroing the whole bank), later chains accumulate
    with start=False. No DVE memsets.
  - Eviction/elementwise ops are emitted on nc.any and load-balanced across
    DVE/ACT by the scheduler.
"""

import numpy as np

import concourse.bass as bass
import concourse.tile as tile
from concourse import bacc, mybir
from concourse.bass_utils import run_bass_kernel_spmd

P = 128
B, C, T, H, W = 2, 512, 64, 32, 32
NCORES = 8
HSH = H // 4          # 8 h-rows per core
CCH = C // P          # 4 c chunks
GRP = 8               # w locations per group (4 pairs)
NGRP = W // GRP       # 4 groups per block
NPAIR = GRP // 2
GCOL = GRP * T        # 512 columns per group
EPS = 1e-6

f32 = mybir.dt.float32
bf16 = mybir.dt.bfloat16
ALU = mybir.AluOpType
AF = mybir.ActivationFunctionType
NPBF16 = mybir.dt.np(bf16)


def build_nc(num_cores=NCORES, nblk=HSH):
    nc = bacc.Bacc("TRN2", target_bir_lowering=False, debug=False,
                   num_devices=num_cores)

    xs = nc.declare_dram_parameter("xs", [C, nblk * W * T], bf16,
                                   isOutput=False)
    gt = nc.declare_dram_parameter("gt", [C, C], bf16, isOutput=False)
    mt = nc.declare_dram_parameter("mt", [C, C], bf16, isOutput=False)
    bcol = nc.declare_dram_parameter("bcol", [P, 2 * CCH], f32, isOutput=False)
    maskp = nc.declare_dram_parameter("maskt", [P, GCOL], bf16, isOutput=False)
    onescol = nc.declare_dram_parameter("onescol", [P, 1], bf16, isOutput=False)
    onesrow = nc.declare_dram_parameter("onesrow", [1, P], bf16, isOutput=False)
    outp = nc.declare_dram_parameter("out", [C, nblk * W * T], bf16,
                                     isOutput=True)

    with tile.TileContext(nc) as tc:
        with (
            tc.tile_pool(name="const", bufs=1) as const,
            tc.tile_pool(name="xpool", bufs=2) as xpool,
            tc.tile_pool(name="ypool", bufs=2) as ypool,
            tc.tile_pool(name="vpool", bufs=2) as vpool,
            tc.tile_pool(name="spool", bufs=2) as spool,
            tc.tile_pool(name="py", bufs=2, space="PSUM") as py,
            tc.tile_pool(name="pav", bufs=2, space="PSUM") as pav,
            tc.tile_pool(name="pss", bufs=2, space="PSUM") as pss,
            tc.tile_pool(name="psm", bufs=1, space="PSUM") as psm,
            tc.tile_pool(name="psx", bufs=1, space="PSUM") as psx,
        ):
            # ---------- constants; only bcol/ones/G gate the first Y
            # matmuls. M and the mask are issued after block 0's x DMA (they
            # are first needed by VT2/softmax, well after Y@0 starts) ----------
            bcol_sb = const.tile([P, 2 * CCH], f32, tag="bcol")
            nc.sync.dma_start(bcol_sb[:], bcol[:])
            oc_sb = const.tile([P, 1], bf16, tag="oc")
            nc.sync.dma_start(oc_sb[:], onescol[:])
            or_sb = const.tile([1, P], bf16, tag="or")
            nc.sync.dma_start(or_sb[:], onesrow[:])
            g_sb, m_sb = [], []
            for ci in range(CCH):
                t = const.tile([P, C], bf16, tag=f"g{ci}")
                nc.sync.dma_start(t[:], gt[ci * P:(ci + 1) * P, :])
                g_sb.append(t)

            def emit_late_consts():
                for ci in range(CCH):
                    t = const.tile([P, C], bf16, tag=f"m{ci}")
                    nc.sync.dma_start(t[:], mt[ci * P:(ci + 1) * P, :])
                    m_sb.append(t)
                t = const.tile([P, GCOL], bf16, tag="maskt")
                nc.sync.dma_start(t[:], maskp[:])
                return t

            with nc.allow_low_precision(reason="bf16 within rel-err budget"):

                def emit_dma_in(blk):
                    xb = []
                    for ci in range(CCH):
                        t = xpool.tile([P, W * T], bf16, tag=f"xb{ci}")
                        nc.sync.dma_start(
                            t[:], xs[ci * P:(ci + 1) * P,
                                     blk * W * T:(blk + 1) * W * T])
                        xb.append(t)
                    return xb

                def emit_a(xb, g):
                    """Y = G x + u projection and (M x)^T per pair."""
                    c0 = g * GCOL
                    yg = []
                    for co in range(CCH):
                        ps = py.tile([P, GCOL], f32, tag="py")
                        for ci in range(CCH):
                            nc.tensor.matmul(
                                ps[:], g_sb[ci][:, co * P:(co + 1) * P],
                                xb[ci][:, c0:c0 + GCOL],
                                start=(ci == 0), stop=(ci == CCH - 1))
                        t = ypool.tile([P, GCOL], bf16, tag=f"y{co}")
                        nc.any.tensor_scalar(t[:], ps[:],
                                             bcol_sb[:, co:co + 1], None,
                                             ALU.add)
                        yg.append(t)
                    vt = []
                    for p in range(NPAIR):
                        ps = pss.tile([P, C], f32, tag="ppv")
                        for ci in range(CCH):
                            nc.tensor.matmul(
                                ps[:], xb[ci][:, c0 + p * P:c0 + (p + 1) * P],
                                m_sb[ci][:],
                                start=(ci == 0), stop=(ci == CCH - 1))
                        t = vpool.tile([P, C], bf16, tag=f"vt{p}")
                        nc.any.tensor_copy(t[:], ps[:])
                        vt.append(t)
                    return yg, vt

                def emit_scores(xb, g, yg):
                    # first matmul start=True zeroes the whole PSUM bank;
                    # later pairs accumulate start=False onto those zeros
                    # (PE executes its queue in program order)
                    c0 = g * GCOL
                    ps_s = psm.tile([P, GCOL], f32, tag="pss")
                    for p in range(NPAIR):
                        for ci in range(CCH):
                            xl = xb[ci][:, c0 + p * P:c0 + (p + 1) * P]
                            nc.tensor.matmul(
                                ps_s[:, p * P:(p + 1) * P], xl,
                                yg[ci][:, p * P:(p + 1) * P],
                                start=(p == 0 and ci == 0),
                                stop=(ci == CCH - 1),
                                skip_group_check=True)
                    return ps_s

                def emit_b(xb, g, ps_s, vt):
                    """Softmax (no max subtraction; scores O(1)), AV (already
                    P-projected via M), cvt bias (softmax weights sum to 1),
                    residual into x in place."""
                    c0 = g * GCOL
                    pexp = spool.tile([P, GCOL], bf16, tag="pexp")
                    nc.scalar.activation(pexp[:], ps_s[:], AF.Exp)
                    pm = spool.tile([P, GCOL], bf16, tag="pmask")
                    nc.any.tensor_mul(pm[:], pexp[:], mask_sb[:])
                    ps_x = psx.tile([P, GCOL], f32, tag="psx")
                    nc.tensor.matmul(ps_x[0:1, :], oc_sb[:], pm[:],
                                     start=True, stop=True)
                    rs = spool.tile([1, GCOL], bf16, tag="rs")
                    nc.vector.reciprocal(rs[:], ps_x[0:1, :])
                    nc.tensor.matmul(ps_x[:], or_sb[:], rs[:],
                                     start=True, stop=True)
                    pn = spool.tile([P, GCOL], bf16, tag="pn")
                    nc.any.tensor_mul(pn[:], pm[:], ps_x[:])
                    for co in range(CCH):
                        ps_o = pav.tile([P, GCOL], f32, tag="pav")
                        for p in range(NPAIR):
                            nc.tensor.matmul(
                                ps_o[:, p * P:(p + 1) * P],
                                vt[p][:, co * P:(co + 1) * P],
                                pn[:, p * P:(p + 1) * P],
                                start=(p == 0), stop=(p == NPAIR - 1),
                                skip_group_check=True)
                        t = spool.tile([P, GCOL], bf16, tag="ot")
                        nc.any.tensor_scalar(
                            t[:], ps_o[:],
                            bcol_sb[:, CCH + co:CCH + co + 1], None, ALU.add)
                        nc.any.tensor_add(xb[co][:, c0:c0 + GCOL], t[:],
                                          xb[co][:, c0:c0 + GCOL])

                # software pipeline: A(next) is emitted between scores(cur)
                # and the softmax-dependent PE work of cur, so the in-order
                # PE queue never blocks on ACT/DVE evictions
                xb_cur = emit_dma_in(0)
                mask_sb = emit_late_consts()
                state = emit_a(xb_cur, 0)
                xb_next = None
                for blk in range(nblk):
                    if blk + 1 < nblk:
                        xb_next = emit_dma_in(blk + 1)
                    for g in range(NGRP):
                        ps_s = emit_scores(xb_cur, g, state[0])
                        vt_cur = state[1]
                        if g + 1 < NGRP:
                            state = emit_a(xb_cur, g + 1)
                        elif blk + 1 < nblk:
                            state = emit_a(xb_next, 0)
                        else:
                            state = None
                        emit_b(xb_cur, g, ps_s, vt_cur)
                    if blk + 1 < nblk:
                        for ci in range(CCH):
                            nc.sync.dma_start(
                                outp[ci * P:(ci + 1) * P,
                                     blk * W * T:(blk + 1) * W * T],
                                xb_cur[ci][:])
                    else:
                        # drain tail: ship the last block per group so the
                        # final DMA after the last compute is only 1/4 block
                        for g in range(NGRP):
                            for ci in range(CCH):
                                o0 = blk * W * T + g * GCOL
                                nc.sync.dma_start(
                                    outp[ci * P:(ci + 1) * P, o0:o0 + GCOL],
                                    xb_cur[ci][:, g * GCOL:(g + 1) * GCOL])
                    xb_cur = xb_next
    nc.compile()
    return nc


def host_prep(x, gamma, beta, wq, bq, wk, bk, wv, bv, wp, bp):
    """Fold GroupNorm stats, gamma/beta, the attention scale, and the Q/K and
    V/P projection pairs into two (c,c) matrices + biases per batch."""
    s = 1.0 / np.sqrt(np.float64(C))
    n = C * T * H * W
    g64 = gamma.astype(np.float64)
    b64 = beta.astype(np.float64)

    per_batch = []
    for b in range(B):
        xf = x[b].reshape(-1)
        s1 = float(xf.sum(dtype=np.float64))
        s2 = float(np.dot(xf, xf))
        mu = s1 / n
        var = s2 / n - mu * mu
        r = 1.0 / np.sqrt(var + EPS)
        gp = g64 * r                       # per-channel scale on x
        cb = b64 - mu * gp                 # per-channel offset

        def fold(w, bias, scale):
            w64 = w.astype(np.float64)
            a = (w64 * gp[None, :]) * scale          # (co, ci)
            c0 = (bias.astype(np.float64) + w64 @ cb) * scale
            return a, c0

        aq, cq = fold(wq, bq, s)
        ak, ck = fold(wk, bk, 1.0)
        av, cv = fold(wv, bv, 1.0)
        wp64 = wp.astype(np.float64)

        G = ak.T @ aq                      # (ci_s -> scores via x^T G x)
        u = ak.T @ cq                      # row bias (varies along s)
        M = wp64 @ av                      # fused V+P projection
        cvt = wp64 @ cv + bp.astype(np.float64)

        bcol = np.empty((P, 2 * CCH), np.float32)
        for ch in range(CCH):
            bcol[:, ch] = u[ch * P:(ch + 1) * P]
            bcol[:, CCH + ch] = cvt[ch * P:(ch + 1) * P]

        per_batch.append({
            "gt": np.ascontiguousarray(G.T).astype(NPBF16),
            "mt": np.ascontiguousarray(M.T).astype(NPBF16),
            "bcol": bcol,
        })

    # mask for one pair block [128, 128]: blockdiag of two causal(64) masks,
    # [s, t] keep s <= t; tiled across the 4 pairs of a group
    tri = np.triu(np.ones((T, T), np.float32))
    blk = np.zeros((P, P), np.float32)
    blk[:T, :T] = tri
    blk[T:, T:] = tri
    maskt = np.tile(blk, (1, NPAIR)).astype(NPBF16)
    shared = {
        "maskt": np.ascontiguousarray(maskt),
        "onescol": np.ones((P, 1), NPBF16),
        "onesrow": np.ones((1, P), NPBF16),
    }
    return per_batch, shared


_NC_CACHE = {}


def kernel(x, gamma, beta, wq, bq, wk, bk, wv, bv, wp, bp):
    x = np.asarray(x, np.float32)
    args = [np.asarray(a, np.float32) for a in
            (gamma, beta, wq, bq, wk, bk, wv, bv, wp, bp)]
    per_batch, shared = host_prep(x, *args)

    if "nc" not in _NC_CACHE:
        _NC_CACHE["nc"] = build_nc()
    nc = _NC_CACHE["nc"]

    in_maps = []
    for core in range(NCORES):
        b, hg = core // 4, core % 4
        shard = x[b, :, :, hg * HSH:(hg + 1) * HSH, :]        # (C,T,HSH,W)
        shard = np.ascontiguousarray(
            shard.transpose(0, 2, 3, 1)).reshape(C, HSH * W * T)
        in_maps.append({"xs": shard.astype(NPBF16),
                        **per_batch[b], **shared})

    global _last_in_maps
    _last_in_maps = in_maps
    res = run_bass_kernel_spmd(nc, in_maps, list(range(NCORES)))

    out = np.empty((B, C, T, H, W), np.float32)
    for core in range(NCORES):
        b, hg = core // 4, core % 4
        o = res.results[core]["out"].astype(np.float32)
        o = o.reshape(C, HSH, W, T)
        out[b, :, :, hg * HSH:(hg + 1) * HSH, :] = o.transpose(0, 3, 1, 2)
    return out


# revision 25
# speedup vs baseline: 1.0373x; 1.0174x over previous
"""CausalTemporalAttnBlock Trainium2 kernel (v3).

Problem: out = x + Wp @ attn(norm(x)) + bp, where norm is GroupNorm(1 group)
over (c,t,h,w) per batch, attention is causal over t, independent per (b,h,w).
Shapes: x (2, 512, 64, 32, 32) fp32; four (512,512) weights + biases.

Strategy (8 NeuronCores, zero device-to-device communication):
  - core i handles batch i//4, h-rows [8*(i%4), 8*(i%4)+8), all w.
  - GroupNorm stats (mean/rstd per batch) are computed on host in float64 and
    folded, with gamma/beta and the attention scale, into the projections.
  - Associativity folding collapses the four (c,c) projections into TWO
    device GEMMs (folds are host-side float64 on (512,512) matrices):
      * scores: k_s . q_t = x_s^T (Ak^T Aq) x_t + (Ak^T cq) . x_s + terms
        that are constant along s and cancel in softmax. So Y = G x + u with
        G = Ak^T Aq, and S^T = x^T Y.
      * output: x + Wp(sum_s a_s v_s) + bp = x + sum_s a_s (M x_s) + cvt,
        with M = Wp Av, cvt = Wp cv + bp (softmax weights sum to 1 exactly).
  - Everything ships and computes in bf16 (PE runs bf16 at 1 cycle/row for
    any moving size, vs fp32r's 4x penalty below 256 columns; DMA bytes and
    SBUF footprint halve). PSUM accumulation stays fp32; biases applied
    during PSUM eviction.
  - Attention processes (h,w) locations in PAIRS packed into the 128-wide
    partition dim: scores for a pair are [128,128] matmuls per ci chunk
    (cross-location junk is zeroed by the mask multiply before the softmax
    column sums), (Mx)^T for a pair is produced at full PE rate with the x
    slab [c,128] stationary, and AV contracts the full 128 partitions.
  - Multi-chain PSUM banks rely on PE program order: the first matmul of a
    bank runs start=True (zeroing the whole bank), later chains accumulate
    with start=False. No DVE memsets.
  - Eviction/elementwise ops are emitted on nc.any and load-balanced across
    DVE/ACT by the scheduler.
"""

import numpy as np

import concourse.bass as bass
import concourse.tile as tile
from concourse import bacc, mybir
from concourse.bass_utils import run_bass_kernel_spmd

P = 128
B, C, T, H, W = 2, 512, 64, 32, 32
NCORES = 8
HSH = H // 4          # 8 h-rows per core
CCH = C // P          # 4 c chunks
GRP = 8               # w locations per group (4 pairs)
NGRP = W // GRP       # 4 groups per block
NPAIR = GRP // 2
GCOL = GRP * T        # 512 columns per group
EPS = 1e-6

f32 = mybir.dt.float32
bf16 = mybir.dt.bfloat16
ALU = mybir.AluOpType
AF = mybir.ActivationFunctionType
NPBF16 = mybir.dt.np(bf16)


def build_nc(num_cores=NCORES, nblk=HSH):
    nc = bacc.Bacc("TRN2", target_bir_lowering=False, debug=False,
                   num_devices=num_cores)

    xs = nc.declare_dram_parameter("xs", [C, nblk * W * T], bf16,
                                   isOutput=False)
    gt = nc.declare_dram_parameter("gt", [C, C], bf16, isOutput=False)
    mt = nc.declare_dram_parameter("mt", [C, C], bf16, isOutput=False)
    bcol = nc.declare_dram_parameter("bcol", [P, 2 * CCH], f32, isOutput=False)
    maskp = nc.declare_dram_parameter("maskt", [P, GCOL], bf16, isOutput=False)
    onescol = nc.declare_dram_parameter("onescol", [P, 1], bf16, isOutput=False)
    onesrow = nc.declare_dram_parameter("onesrow", [1, P], bf16, isOutput=False)
    outp = nc.declare_dram_parameter("out", [C, nblk * W * T], bf16,
                                     isOutput=True)

    with tile.TileContext(nc) as tc:
        with (
            tc.tile_pool(name="const", bufs=1) as const,
            tc.tile_pool(name="xpool", bufs=2) as xpool,
            tc.tile_pool(name="ypool", bufs=2) as ypool,
            tc.tile_pool(name="vpool", bufs=2) as vpool,
            tc.tile_pool(name="spool", bufs=2) as spool,
            tc.tile_pool(name="py", bufs=3, space="PSUM") as py,
            tc.tile_pool(name="pav", bufs=2, space="PSUM") as pav,
            tc.tile_pool(name="pss", bufs=1, space="PSUM") as pss,
            tc.tile_pool(name="psm", bufs=1, space="PSUM") as psm,
            tc.tile_pool(name="psx", bufs=1, space="PSUM") as psx,
        ):
            # ---------- constants; only bcol/ones/G gate the first Y
            # matmuls. M and the mask are issued after block 0's x DMA (they
            # are first needed by VT2/softmax, well after Y@0 starts) ----------
            bcol_sb = const.tile([P, 2 * CCH], f32, tag="bcol")
            nc.sync.dma_start(bcol_sb[:], bcol[:])
            oc_sb = const.tile([P, 1], bf16, tag="oc")
            nc.sync.dma_start(oc_sb[:], onescol[:])
            or_sb = const.tile([1, P], bf16, tag="or")
            nc.sync.dma_start(or_sb[:], onesrow[:])
            g_sb, m_sb = [], []
            for ci in range(CCH):
                t = const.tile([P, C], bf16, tag=f"g{ci}")
                nc.sync.dma_start(t[:], gt[ci * P:(ci + 1) * P, :])
                g_sb.append(t)

            def emit_late_consts():
                for ci in range(CCH):
                    t = const.tile([P, C], bf16, tag=f"m{ci}")
                    nc.sync.dma_start(t[:], mt[ci * P:(ci + 1) * P, :])
                    m_sb.append(t)
                t = const.tile([P, GCOL], bf16, tag="maskt")
                nc.sync.dma_start(t[:], maskp[:])
                return t

            with nc.allow_low_precision(reason="bf16 within rel-err budget"):

                def emit_dma_in(blk, split=False):
                    # split=True (block 0 only): per-group sub-DMAs so the
                    # first Y matmuls gate on ~1MB instead of the whole block
                    xb = []
                    for ci in range(CCH):
                        t = xpool.tile([P, W * T], bf16, tag=f"xb{ci}")
                        if split:
                            for g in range(NGRP):
                                nc.sync.dma_start(
                                    t[:, g * GCOL:(g + 1) * GCOL],
                                    xs[ci * P:(ci + 1) * P,
                                       blk * W * T + g * GCOL:
                                       blk * W * T + (g + 1) * GCOL])
                        else:
                            nc.sync.dma_start(
                                t[:], xs[ci * P:(ci + 1) * P,
                                         blk * W * T:(blk + 1) * W * T])
                        xb.append(t)
                    return xb

                def emit_a(xb, g):
                    """Y = G x + u projection and (M x)^T per pair."""
                    c0 = g * GCOL
                    yg = []
                    for co in range(CCH):
                        ps = py.tile([P, GCOL], f32, tag="py")
                        for ci in range(CCH):
                            nc.tensor.matmul(
                                ps[:], g_sb[ci][:, co * P:(co + 1) * P],
                                xb[ci][:, c0:c0 + GCOL],
                                start=(ci == 0), stop=(ci == CCH - 1))
                        t = ypool.tile([P, GCOL], bf16, tag=f"y{co}")
                        nc.any.tensor_scalar(t[:], ps[:],
                                             bcol_sb[:, co:co + 1], None,
                                             ALU.add)
                        yg.append(t)
                    vt = []
                    for p in range(NPAIR):
                        ps = pss.tile([P, C], f32, tag="ppv")
                        for ci in range(CCH):
                            nc.tensor.matmul(
                                ps[:], xb[ci][:, c0 + p * P:c0 + (p + 1) * P],
                                m_sb[ci][:],
                                start=(ci == 0), stop=(ci == CCH - 1))
                        t = vpool.tile([P, C], bf16, tag=f"vt{p}")
                        nc.any.tensor_copy(t[:], ps[:])
                        vt.append(t)
                    return yg, vt

                def emit_scores(xb, g, yg):
                    # first matmul start=True zeroes the whole PSUM bank;
                    # later pairs accumulate start=False onto those zeros
                    # (PE executes its queue in program order)
                    c0 = g * GCOL
                    ps_s = psm.tile([P, GCOL], f32, tag="pss")
                    for p in range(NPAIR):
                        for ci in range(CCH):
                            xl = xb[ci][:, c0 + p * P:c0 + (p + 1) * P]
                            nc.tensor.matmul(
                                ps_s[:, p * P:(p + 1) * P], xl,
                                yg[ci][:, p * P:(p + 1) * P],
                                start=(p == 0 and ci == 0),
                                stop=(ci == CCH - 1),
                                skip_group_check=True)
                    return ps_s

                def emit_b(xb, g, ps_s, vt):
                    """Softmax (no max subtraction; scores O(1)), AV (already
                    P-projected via M), cvt bias (softmax weights sum to 1),
                    residual into x in place."""
                    c0 = g * GCOL
                    pexp = spool.tile([P, GCOL], bf16, tag="pexp")
                    nc.scalar.activation(pexp[:], ps_s[:], AF.Exp)
                    pm = spool.tile([P, GCOL], bf16, tag="pmask")
                    nc.any.tensor_mul(pm[:], pexp[:], mask_sb[:])
                    ps_x = psx.tile([P, GCOL], f32, tag="psx")
                    nc.tensor.matmul(ps_x[0:1, :], oc_sb[:], pm[:],
                                     start=True, stop=True)
                    rs = spool.tile([1, GCOL], bf16, tag="rs")
                    nc.vector.reciprocal(rs[:], ps_x[0:1, :])
                    nc.tensor.matmul(ps_x[:], or_sb[:], rs[:],
                                     start=True, stop=True)
                    pn = spool.tile([P, GCOL], bf16, tag="pn")
                    nc.any.tensor_mul(pn[:], pm[:], ps_x[:])
                    for co in range(CCH):
                        ps_o = pav.tile([P, GCOL], f32, tag="pav")
                        for p in range(NPAIR):
                            nc.tensor.matmul(
                                ps_o[:, p * P:(p + 1) * P],
                                vt[p][:, co * P:(co + 1) * P],
                                pn[:, p * P:(p + 1) * P],
                                start=(p == 0), stop=(p == NPAIR - 1),
                                skip_group_check=True)
                        t = spool.tile([P, GCOL], bf16, tag="ot")
                        nc.any.tensor_scalar(
                            t[:], ps_o[:],
                            bcol_sb[:, CCH + co:CCH + co + 1], None, ALU.add)
                        nc.any.tensor_add(xb[co][:, c0:c0 + GCOL], t[:],
                                          xb[co][:, c0:c0 + GCOL])

                # software pipeline: A(next) is emitted between scores(cur)
                # and the softmax-dependent PE work of cur, so the in-order
                # PE queue never blocks on ACT/DVE evictions
                xb_cur = emit_dma_in(0)
                mask_sb = emit_late_consts()
                state = emit_a(xb_cur, 0)
                xb_next = None
                for blk in range(nblk):
                    if blk + 1 < nblk:
                        xb_next = emit_dma_in(blk + 1)
                    for g in range(NGRP):
                        ps_s = emit_scores(xb_cur, g, state[0])
                        vt_cur = state[1]
                        if g + 1 < NGRP:
                            state = emit_a(xb_cur, g + 1)
                        elif blk + 1 < nblk:
                            state = emit_a(xb_next, 0)
                        else:
                            state = None
                        emit_b(xb_cur, g, ps_s, vt_cur)
                    if blk + 1 < nblk:
                        for ci in range(CCH):
                            nc.sync.dma_start(
                                outp[ci * P:(ci + 1) * P,
                                     blk * W * T:(blk + 1) * W * T],
                                xb_cur[ci][:])
                    else:
                        # drain tail: ship the last block per group so the
                        # final DMA after the last compute is only 1/4 block
                        for g in range(NGRP):
                            for ci in range(CCH):
                                o0 = blk * W * T + g * GCOL
                                nc.sync.dma_start(
                                    outp[ci * P:(ci + 1) * P, o0:o0 + GCOL],
                                    xb_cur[ci][:, g * GCOL:(g + 1) * GCOL])
                    xb_cur = xb_next
    nc.compile()
    return nc


def host_prep(x, gamma, beta, wq, bq, wk, bk, wv, bv, wp, bp):
    """Fold GroupNorm stats, gamma/beta, the attention scale, and the Q/K and
    V/P projection pairs into two (c,c) matrices + biases per batch."""
    s = 1.0 / np.sqrt(np.float64(C))
    n = C * T * H * W
    g64 = gamma.astype(np.float64)
    b64 = beta.astype(np.float64)

    per_batch = []
    for b in range(B):
        xf = x[b].reshape(-1)
        s1 = float(xf.sum(dtype=np.float64))
        s2 = float(np.dot(xf, xf))
        mu = s1 / n
        var = s2 / n - mu * mu
        r = 1.0 / np.sqrt(var + EPS)
        gp = g64 * r                       # per-channel scale on x
        cb = b64 - mu * gp                 # per-channel offset

        def fold(w, bias, scale):
            w64 = w.astype(np.float64)
            a = (w64 * gp[None, :]) * scale          # (co, ci)
            c0 = (bias.astype(np.float64) + w64 @ cb) * scale
            return a, c0

        aq, cq = fold(wq, bq, s)
        ak, ck = fold(wk, bk, 1.0)
        av, cv = fold(wv, bv, 1.0)
        wp64 = wp.astype(np.float64)

        G = ak.T @ aq                      # (ci_s -> scores via x^T G x)
        u = ak.T @ cq                      # row bias (varies along s)
        M = wp64 @ av                      # fused V+P projection
        cvt = wp64 @ cv + bp.astype(np.float64)

        bcol = np.empty((P, 2 * CCH), np.float32)
        for ch in range(CCH):
            bcol[:, ch] = u[ch * P:(ch + 1) * P]
            bcol[:, CCH + ch] = cvt[ch * P:(ch + 1) * P]

        per_batch.append({
            "gt": np.ascontiguousarray(G.T).astype(NPBF16),
            "mt": np.ascontiguousarray(M.T).astype(NPBF16),
            "bcol": bcol,
        })

    # mask for one pair block [128, 128]: blockdiag of two causal(64) masks,
    # [s, t] keep s <= t; tiled across the 4 pairs of a group
    tri = np.triu(np.ones((T, T), np.float32))
    blk = np.zeros((P, P), np.float32)
    blk[:T, :T] = tri
    blk[T:, T:] = tri
    maskt = np.tile(blk, (1, NPAIR)).astype(NPBF16)
    shared = {
        "maskt": np.ascontiguousarray(maskt),
        "onescol": np.ones((P, 1), NPBF16),
        "onesrow": np.ones((1, P), NPBF16),
    }
    return per_batch, shared


_NC_CACHE = {}


def kernel(x, gamma, beta, wq, bq, wk, bk, wv, bv, wp, bp):
    x = np.asarray(x, np.float32)
    args = [np.asarray(a, np.float32) for a in
            (gamma, beta, wq, bq, wk, bk, wv, bv, wp, bp)]
    per_batch, shared = host_prep(x, *args)

    if "nc" not in _NC_CACHE:
        _NC_CACHE["nc"] = build_nc()
    nc = _NC_CACHE["nc"]

    in_maps = []
    for core in range(NCORES):
        b, hg = core // 4, core % 4
        shard = x[b, :, :, hg * HSH:(hg + 1) * HSH, :]        # (C,T,HSH,W)
        shard = np.ascontiguousarray(
            shard.transpose(0, 2, 3, 1)).reshape(C, HSH * W * T)
        in_maps.append({"xs": shard.astype(NPBF16),
                        **per_batch[b], **shared})

    global _last_in_maps
    _last_in_maps = in_maps
    res = run_bass_kernel_spmd(nc, in_maps, list(range(NCORES)))

    out = np.empty((B, C, T, H, W), np.float32)
    for core in range(NCORES):
        b, hg = core // 4, core % 4
        o = res.results[core]["out"].astype(np.float32)
        o = o.reshape(C, HSH, W, T)
        out[b, :, :, hg * HSH:(hg + 1) * HSH, :] = o.transpose(0, 3, 1, 2)
    return out


# revision 26
# speedup vs baseline: 1.0860x; 1.0470x over previous
"""CausalTemporalAttnBlock Trainium2 kernel (v3).

Problem: out = x + Wp @ attn(norm(x)) + bp, where norm is GroupNorm(1 group)
over (c,t,h,w) per batch, attention is causal over t, independent per (b,h,w).
Shapes: x (2, 512, 64, 32, 32) fp32; four (512,512) weights + biases.

Strategy (8 NeuronCores, zero device-to-device communication):
  - core i handles batch i//4, h-rows [8*(i%4), 8*(i%4)+8), all w.
  - GroupNorm stats (mean/rstd per batch) are computed on host in float64 and
    folded, with gamma/beta and the attention scale, into the projections.
  - Associativity folding collapses the four (c,c) projections into TWO
    device GEMMs (folds are host-side float64 on (512,512) matrices):
      * scores: k_s . q_t = x_s^T (Ak^T Aq) x_t + (Ak^T cq) . x_s + terms
        that are constant along s and cancel in softmax. So Y = G x + u with
        G = Ak^T Aq, and S^T = x^T Y.
      * output: x + Wp(sum_s a_s v_s) + bp = x + sum_s a_s (M x_s) + cvt,
        with M = Wp Av, cvt = Wp cv + bp (softmax weights sum to 1 exactly).
  - Everything ships and computes in bf16 (PE runs bf16 at 1 cycle/row for
    any moving size, vs fp32r's 4x penalty below 256 columns; DMA bytes and
    SBUF footprint halve). PSUM accumulation stays fp32; biases applied
    during PSUM eviction.
  - Attention processes (h,w) locations in PAIRS packed into the 128-wide
    partition dim: scores for a pair are [128,128] matmuls per ci chunk
    (cross-location junk is zeroed by the mask multiply before the softmax
    column sums), (Mx)^T for a pair is produced at full PE rate with the x
    slab [c,128] stationary, and AV contracts the full 128 partitions.
  - Multi-chain PSUM banks rely on PE program order: the first matmul of a
    bank runs start=True (zeroing the whole bank), later chains accumulate
    with start=False. No DVE memsets.
  - Eviction/elementwise ops are emitted on nc.any and load-balanced across
    DVE/ACT by the scheduler.
"""

import numpy as np

import concourse.bass as bass
import concourse.tile as tile
from concourse import bacc, bass_isa, mybir
from concourse.bass_utils import run_bass_kernel_spmd

P = 128
B, C, T, H, W = 2, 512, 64, 32, 32
NCORES = 8
HSH = H // 4          # 8 h-rows per core
CCH = C // P          # 4 c chunks
GRP = 8               # w locations per group (4 pairs)
NGRP = W // GRP       # 4 groups per block
NPAIR = GRP // 2
GCOL = GRP * T        # 512 columns per group
EPS = 1e-6

f32 = mybir.dt.float32
bf16 = mybir.dt.bfloat16
ALU = mybir.AluOpType
AF = mybir.ActivationFunctionType
NPBF16 = mybir.dt.np(bf16)


def build_nc(num_cores=NCORES, nblk=HSH):
    nc = bacc.Bacc("TRN2", target_bir_lowering=False, debug=False,
                   num_devices=num_cores)

    xs = nc.declare_dram_parameter("xs", [C, nblk * W * T], bf16,
                                   isOutput=False)
    gt = nc.declare_dram_parameter("gt", [C, C], bf16, isOutput=False)
    mt = nc.declare_dram_parameter("mt", [C, C], bf16, isOutput=False)
    bcol = nc.declare_dram_parameter("bcol", [P, 2 * CCH], f32, isOutput=False)
    maskp = nc.declare_dram_parameter("maskt", [P, GCOL], bf16, isOutput=False)
    onescol = nc.declare_dram_parameter("onescol", [P, 1], bf16, isOutput=False)
    onesrow = nc.declare_dram_parameter("onesrow", [1, P], bf16, isOutput=False)
    outp = nc.declare_dram_parameter("out", [C, nblk * W * T], bf16,
                                     isOutput=True)

    with tile.TileContext(nc) as tc:
        with (
            tc.tile_pool(name="const", bufs=1) as const,
            tc.tile_pool(name="xpool", bufs=2) as xpool,
            tc.tile_pool(name="ypool", bufs=2) as ypool,
            tc.tile_pool(name="vpool", bufs=2) as vpool,
            tc.tile_pool(name="spool", bufs=2) as spool,
            tc.tile_pool(name="py", bufs=4, space="PSUM") as py,
            tc.tile_pool(name="pav", bufs=2, space="PSUM") as pav,
            tc.tile_pool(name="pss", bufs=1, space="PSUM") as pss,
            tc.tile_pool(name="psm", bufs=1, space="PSUM") as psm,
        ):
            # ---------- constants; only bcol/ones/G gate the first Y
            # matmuls. M and the mask are issued after block 0's x DMA (they
            # are first needed by VT2/softmax, well after Y@0 starts) ----------
            bcol_sb = const.tile([P, 2 * CCH], f32, tag="bcol")
            nc.sync.dma_start(bcol_sb[:], bcol[:])
            oc_sb = const.tile([P, 1], bf16, tag="oc")
            nc.sync.dma_start(oc_sb[:], onescol[:])
            or_sb = const.tile([1, P], bf16, tag="or")
            nc.sync.dma_start(or_sb[:], onesrow[:])
            g_sb, m_sb = [], []
            for ci in range(CCH):
                t = const.tile([P, C], bf16, tag=f"g{ci}")
                nc.sync.dma_start(t[:], gt[ci * P:(ci + 1) * P, :])
                g_sb.append(t)

            def emit_late_consts():
                for ci in range(CCH):
                    t = const.tile([P, C], bf16, tag=f"m{ci}")
                    nc.sync.dma_start(t[:], mt[ci * P:(ci + 1) * P, :])
                    m_sb.append(t)
                t = const.tile([P, GCOL], bf16, tag="maskt")
                nc.sync.dma_start(t[:], maskp[:])
                return t

            with nc.allow_low_precision(reason="bf16 within rel-err budget"):

                def emit_dma_in(blk, split=False):
                    # split=True (block 0 only): per-group sub-DMAs so the
                    # first Y matmuls gate on ~1MB instead of the whole block
                    xb = []
                    for ci in range(CCH):
                        t = xpool.tile([P, W * T], bf16, tag=f"xb{ci}")
                        if split:
                            for g in range(NGRP):
                                nc.sync.dma_start(
                                    t[:, g * GCOL:(g + 1) * GCOL],
                                    xs[ci * P:(ci + 1) * P,
                                       blk * W * T + g * GCOL:
                                       blk * W * T + (g + 1) * GCOL])
                        else:
                            nc.sync.dma_start(
                                t[:], xs[ci * P:(ci + 1) * P,
                                         blk * W * T:(blk + 1) * W * T])
                        xb.append(t)
                    return xb

                def emit_a(xb, g):
                    """Y = G x + u projection and (M x)^T per pair."""
                    c0 = g * GCOL
                    yg = []
                    for co in range(CCH):
                        ps = py.tile([P, GCOL], f32, tag="py")
                        for ci in range(CCH):
                            nc.tensor.matmul(
                                ps[:], g_sb[ci][:, co * P:(co + 1) * P],
                                xb[ci][:, c0:c0 + GCOL],
                                start=(ci == 0), stop=(ci == CCH - 1))
                        t = ypool.tile([P, GCOL], bf16, tag=f"y{co}")
                        nc.any.tensor_scalar(t[:], ps[:],
                                             bcol_sb[:, co:co + 1], None,
                                             ALU.add)
                        yg.append(t)
                    vt = []
                    for p in range(NPAIR):
                        ps = pss.tile([P, C], f32, tag="ppv")
                        for ci in range(CCH):
                            nc.tensor.matmul(
                                ps[:], xb[ci][:, c0 + p * P:c0 + (p + 1) * P],
                                m_sb[ci][:],
                                start=(ci == 0), stop=(ci == CCH - 1))
                        t = vpool.tile([P, C], bf16, tag=f"vt{p}")
                        nc.any.tensor_copy(t[:], ps[:])
                        vt.append(t)
                    return yg, vt

                def emit_scores(xb, g, yg):
                    # first matmul start=True zeroes the whole PSUM bank;
                    # later pairs accumulate start=False onto those zeros
                    # (PE executes its queue in program order)
                    c0 = g * GCOL
                    ps_s = psm.tile([P, GCOL], f32, tag="pss")
                    for p in range(NPAIR):
                        for ci in range(CCH):
                            xl = xb[ci][:, c0 + p * P:c0 + (p + 1) * P]
                            nc.tensor.matmul(
                                ps_s[:, p * P:(p + 1) * P], xl,
                                yg[ci][:, p * P:(p + 1) * P],
                                start=(p == 0 and ci == 0),
                                stop=(ci == CCH - 1),
                                skip_group_check=True)
                    return ps_s

                def emit_b(xb, g, ps_s, vt):
                    """Softmax (no max subtraction; scores O(1)), AV (already
                    P-projected via M), cvt bias (softmax weights sum to 1),
                    residual into x in place."""
                    c0 = g * GCOL
                    pexp = spool.tile([P, GCOL], bf16, tag="pexp")
                    nc.scalar.activation(pexp[:], ps_s[:], AF.Exp)
                    pm = spool.tile([P, GCOL], bf16, tag="pmask")
                    nc.any.tensor_mul(pm[:], pexp[:], mask_sb[:])
                    # softmax sums on the otherwise-idle gpsimd engine: every
                    # partition receives the column sum, so no PE sum/bcast
                    # matmuls and no PSUM scratch bank
                    sums = spool.tile([P, GCOL], f32, tag="sums")
                    nc.gpsimd.partition_all_reduce(sums[:], pm[:], P,
                                                   bass_isa.ReduceOp.add)
                    srec = spool.tile([P, GCOL], bf16, tag="srec")
                    nc.vector.reciprocal(srec[:], sums[:])
                    pn = spool.tile([P, GCOL], bf16, tag="pn")
                    nc.any.tensor_mul(pn[:], pm[:], srec[:])
                    for co in range(CCH):
                        ps_o = pav.tile([P, GCOL], f32, tag="pav")
                        for p in range(NPAIR):
                            nc.tensor.matmul(
                                ps_o[:, p * P:(p + 1) * P],
                                vt[p][:, co * P:(co + 1) * P],
                                pn[:, p * P:(p + 1) * P],
                                start=(p == 0), stop=(p == NPAIR - 1),
                                skip_group_check=True)
                        t = spool.tile([P, GCOL], bf16, tag="ot")
                        nc.any.tensor_scalar(
                            t[:], ps_o[:],
                            bcol_sb[:, CCH + co:CCH + co + 1], None, ALU.add)
                        nc.any.tensor_add(xb[co][:, c0:c0 + GCOL], t[:],
                                          xb[co][:, c0:c0 + GCOL])

                # software pipeline: A(next) is emitted between scores(cur)
                # and the softmax-dependent PE work of cur, so the in-order
                # PE queue never blocks on ACT/DVE evictions
                xb_cur = emit_dma_in(0)
                mask_sb = emit_late_consts()
                state = emit_a(xb_cur, 0)
                xb_next = None
                for blk in range(nblk):
                    if blk + 1 < nblk:
                        xb_next = emit_dma_in(blk + 1)
                    for g in range(NGRP):
                        ps_s = emit_scores(xb_cur, g, state[0])
                        vt_cur = state[1]
                        if g + 1 < NGRP:
                            state = emit_a(xb_cur, g + 1)
                        elif blk + 1 < nblk:
                            state = emit_a(xb_next, 0)
                        else:
                            state = None
                        emit_b(xb_cur, g, ps_s, vt_cur)
                    if blk + 1 < nblk:
                        for ci in range(CCH):
                            nc.sync.dma_start(
                                outp[ci * P:(ci + 1) * P,
                                     blk * W * T:(blk + 1) * W * T],
                                xb_cur[ci][:])
                    else:
                        # drain tail: ship the last block per group so the
                        # final DMA after the last compute is only 1/4 block
                        for g in range(NGRP):
                            for ci in range(CCH):
                                o0 = blk * W * T + g * GCOL
                                nc.sync.dma_start(
                                    outp[ci * P:(ci + 1) * P, o0:o0 + GCOL],
                                    xb_cur[ci][:, g * GCOL:(g + 1) * GCOL])
                    xb_cur = xb_next
    nc.compile()
    return nc


def host_prep(x, gamma, beta, wq, bq, wk, bk, wv, bv, wp, bp):
    """Fold GroupNorm stats, gamma/beta, the attention scale, and the Q/K and
    V/P projection pairs into two (c,c) matrices + biases per batch."""
    s = 1.0 / np.sqrt(np.float64(C))
    n = C * T * H * W
    g64 = gamma.astype(np.float64)
    b64 = beta.astype(np.float64)

    per_batch = []
    for b in range(B):
        xf = x[b].reshape(-1)
        s1 = float(xf.sum(dtype=np.float64))
        s2 = float(np.dot(xf, xf))
        mu = s1 / n
        var = s2 / n - mu * mu
        r = 1.0 / np.sqrt(var + EPS)
        gp = g64 * r                       # per-channel scale on x
        cb = b64 - mu * gp                 # per-channel offset

        def fold(w, bias, scale):
            w64 = w.astype(np.float64)
            a = (w64 * gp[None, :]) * scale          # (co, ci)
            c0 = (bias.astype(np.float64) + w64 @ cb) * scale
            return a, c0

        aq, cq = fold(wq, bq, s)
        ak, ck = fold(wk, bk, 1.0)
        av, cv = fold(wv, bv, 1.0)
        wp64 = wp.astype(np.float64)

        G = ak.T @ aq                      # (ci_s -> scores via x^T G x)
        u = ak.T @ cq                      # row bias (varies along s)
        M = wp64 @ av                      # fused V+P projection
        cvt = wp64 @ cv + bp.astype(np.float64)

        bcol = np.empty((P, 2 * CCH), np.float32)
        for ch in range(CCH):
            bcol[:, ch] = u[ch * P:(ch + 1) * P]
            bcol[:, CCH + ch] = cvt[ch * P:(ch + 1) * P]

        per_batch.append({
            "gt": np.ascontiguousarray(G.T).astype(NPBF16),
            "mt": np.ascontiguousarray(M.T).astype(NPBF16),
            "bcol": bcol,
        })

    # mask for one pair block [128, 128]: blockdiag of two causal(64) masks,
    # [s, t] keep s <= t; tiled across the 4 pairs of a group
    tri = np.triu(np.ones((T, T), np.float32))
    blk = np.zeros((P, P), np.float32)
    blk[:T, :T] = tri
    blk[T:, T:] = tri
    maskt = np.tile(blk, (1, NPAIR)).astype(NPBF16)
    shared = {
        "maskt": np.ascontiguousarray(maskt),
        "onescol": np.ones((P, 1), NPBF16),
        "onesrow": np.ones((1, P), NPBF16),
    }
    return per_batch, shared


_NC_CACHE = {}


def kernel(x, gamma, beta, wq, bq, wk, bk, wv, bv, wp, bp):
    x = np.asarray(x, np.float32)
    args = [np.asarray(a, np.float32) for a in
            (gamma, beta, wq, bq, wk, bk, wv, bv, wp, bp)]
    per_batch, shared = host_prep(x, *args)

    if "nc" not in _NC_CACHE:
        _NC_CACHE["nc"] = build_nc()
    nc = _NC_CACHE["nc"]

    in_maps = []
    for core in range(NCORES):
        b, hg = core // 4, core % 4
        shard = x[b, :, :, hg * HSH:(hg + 1) * HSH, :]        # (C,T,HSH,W)
        shard = np.ascontiguousarray(
            shard.transpose(0, 2, 3, 1)).reshape(C, HSH * W * T)
        in_maps.append({"xs": shard.astype(NPBF16),
                        **per_batch[b], **shared})

    global _last_in_maps
    _last_in_maps = in_maps
    res = run_bass_kernel_spmd(nc, in_maps, list(range(NCORES)))

    out = np.empty((B, C, T, H, W), np.float32)
    for core in range(NCORES):
        b, hg = core // 4, core % 4
        o = res.results[core]["out"].astype(np.float32)
        o = o.reshape(C, HSH, W, T)
        out[b, :, :, hg * HSH:(hg + 1) * HSH, :] = o.transpose(0, 3, 1, 2)
    return out
